# revision 3
# baseline (speedup 1.0000x reference)
"""Trainium2 Bass kernel — matmul-interpolation v3 (quarter-packed).

v2 was DMA-bound (88%): 71MB of weight columns/core because each point
uploaded 4 columns (y/z samples x two wx-scaled copies for the two
x-columns).  v3 packs the stationary as k = (64 y-rows x 2 x-cols), so
ONE 256B column holds all 4 bilinear corners (wy*wx products) of a
sample: M bytes halve and interp matmul count halves.  The 16MB
SBUF-resident table is replaced by per-group streamed quarter tiles
Q[x0, w] (128x128 fp16, w = 64-row y-window), concatenated with the
group's M blocks into one DMA segment (two groups per DMA).

Binning: ybin/zbin in 0..6: even bins 2w = interior of window w
(row0 % 64 <= 62), odd bins = window-straddling rows (row0 % 64 == 63)
whose two corners split across windows w and w+1 (the point gets one
column in each window's block).  Points sorted by (x0, ybin, zbin);
per-(group, 7x7-cell) counts padded to the max over the 8 cores.

PSUM accumulation-group semantics (measured): the FIRST start=True
matmul arms the whole 2KB region; later start=False matmuls read
logical zero for unwritten bytes; a second start=True would RE-ARM and
discard earlier sums — so exactly one start=True per psum tile.
"""

import numpy as np

import concourse.bass as bass
import concourse.bacc as bacc
import concourse.mybir as mybir
import concourse.tile as tile

N_CORES = 8
N_TOTAL = 500_000
NPC = N_TOTAL // N_CORES
NG = 255                      # groups: x0 in 0..254
NB = 7                        # y/z bins
WIN = 512                     # MLP window / psum bank slots
NW = 4                        # 64-row y-windows

F32 = mybir.dt.float32
F16 = mybir.dt.float16


def point_bins(pts):
    ix = (pts[:, 0] + np.float32(1)) * np.float32(127.5)
    iy = (pts[:, 1] + np.float32(1)) * np.float32(127.5)
    iz = (pts[:, 2] + np.float32(1)) * np.float32(127.5)
    x0 = np.clip(np.floor(ix), 0, 254).astype(np.int32)
    y0 = np.clip(np.floor(iy), 0, 254).astype(np.int32)
    z0 = np.clip(np.floor(iz), 0, 254).astype(np.int32)
    wx1 = (ix - x0).astype(np.float32); wx0 = np.float32(1) - wx1
    wy1 = (iy - y0).astype(np.float32); wy0 = np.float32(1) - wy1
    wz1 = (iz - z0).astype(np.float32); wz0 = np.float32(1) - wz1
    yq, yr = np.divmod(y0, 64)
    zq, zr = np.divmod(z0, 64)
    ybin = (2 * yq + (yr == 63)).astype(np.int32)
    zbin = (2 * zq + (zr == 63)).astype(np.int32)
    return dict(x0=x0, y0=y0, z0=z0, wx0=wx0, wx1=wx1, wy0=wy0, wy1=wy1,
                wz0=wz0, wz1=wz1, ybin=ybin, zbin=zbin,
                cell=ybin * NB + zbin)


def core_counts(b):
    cnt = np.zeros((NG, NB * NB), np.int64)
    np.add.at(cnt, (b["x0"], b["cell"]), 1)
    return cnt


def win_bins(w):
    """bins whose corners intersect window w."""
    return max(2 * w - 1, 0), min(2 * w + 1, 2 * NW - 2)


class Layout:
    def __init__(self, caps):
        self.caps = caps                           # [NG, 49]
        self.cell_off = np.zeros((NG, NB * NB + 1), np.int64)
        self.cell_off[:, 1:] = np.cumsum(caps, axis=1)
        self.S_g = self.cell_off[:, -1].copy()
        assert self.S_g.max() <= WIN, self.S_g.max()
        self.slot_off = np.zeros(NG + 1, np.int64)
        self.slot_off[1:] = np.cumsum(self.S_g)
        self.S_total = int(self.slot_off[-1])
        self.n_win = (self.S_total + WIN - 1) // WIN
        self.out_len = self.n_win * WIN

        c3 = caps.reshape(NG, NB, NB)
        # y-block (window w): slots [cell_off[blo*NB], cell_off[(bhi+1)*NB])
        self.ybw = np.zeros((NG, NW), np.int64)    # widths
        self.ylo = np.zeros((NG, NW), np.int64)    # slot range lo
        self.yhi = np.zeros((NG, NW), np.int64)
        for w in range(NW):
            blo, bhi = win_bins(w)
            self.ylo[:, w] = self.cell_off[:, blo * NB]
            self.yhi[:, w] = self.cell_off[:, (bhi + 1) * NB]
            self.ybw[:, w] = self.yhi[:, w] - self.ylo[:, w]
        # z-block (window w): runs per ybin row r: cells [r*NB+blo, r*NB+bhi+1)
        self.zbw = np.zeros((NG, NW), np.int64)
        self.zrun_off = np.zeros((NG, NW, NB + 1), np.int64)  # within block
        for w in range(NW):
            blo, bhi = win_bins(w)
            rw = c3[:, :, blo:bhi + 1].sum(2)      # [NG, NB] run widths
            self.zrun_off[:, w, 1:] = np.cumsum(rw, axis=1)
            self.zbw[:, w] = self.zrun_off[:, w, -1]
        # windows needed per group
        self.need = (self.ybw > 0) | (self.zbw > 0)      # [NG, NW]
        self.nw = self.need.sum(1)
        # segment layout per group: [stat tiles | y blocks | z blocks]
        self.stat_cols = self.nw * 128
        self.m_cols = self.ybw.sum(1) + self.zbw.sum(1)
        self.seg_w = self.stat_cols + self.m_cols
        self.seg_off = np.zeros(NG + 1, np.int64)
        self.seg_off[1:] = np.cumsum(self.seg_w)
        self.TOT = int(self.seg_off[-1])
        # block offsets within segment
        self.yblk_off = np.zeros((NG, NW), np.int64)
        self.zblk_off = np.zeros((NG, NW), np.int64)
        for g in range(NG):
            off = int(self.stat_cols[g])
            for w in range(NW):
                self.yblk_off[g, w] = off; off += int(self.ybw[g, w])
            for w in range(NW):
                self.zblk_off[g, w] = off; off += int(self.zbw[g, w])
        # window index within stat area
        self.wslot = np.full((NG, NW), -1, np.int64)
        for g in range(NG):
            i = 0
            for w in range(NW):
                if self.need[g, w]:
                    self.wslot[g, w] = i; i += 1
        self.seg_max_pair = int(max(
            self.seg_w[g] + (self.seg_w[g + 1] if g + 1 < NG else 0)
            for g in range(0, NG, 2)))

    def key(self):
        return self.caps.tobytes()


def build_quarters(plane):
    """Q[c, w] = [128 rows = 2*y'+col, 128 ch] fp16, rows from window w."""
    t = np.ascontiguousarray(plane.transpose(1, 2, 0)).astype(np.float16)  # [y, x, ch]
    sw = np.lib.stride_tricks.sliding_window_view(t, 2, axis=1)  # [y, 255, ch, 2]
    # want Q[c, w, 2*y'+col, ch] = t[64w+y', c+col, ch]
    q = sw.reshape(NW, 64, NG, 128, 2)            # [w, y', c, ch, col]
    q = q.transpose(2, 0, 1, 4, 3)                # [c, w, y', col, ch]
    return np.ascontiguousarray(q).reshape(NG, NW, 128, 128)


def prep_shared(inputs):
    plane = np.asarray(inputs["plane_xy"], np.float32)[0]
    return {
        "Q": build_quarters(plane),
        "w1t": np.ascontiguousarray(np.asarray(inputs["W1"], np.float32).T).astype(np.float16),
        "w2t": np.ascontiguousarray(np.asarray(inputs["W2"], np.float32).T).astype(np.float16),
        "w3t": np.ascontiguousarray(np.asarray(inputs["W3"], np.float32).T).astype(np.float16),
        "b1s": (np.float32(30.0) * np.asarray(inputs["b1"], np.float32)).reshape(128, 1),
        "b2s": (np.float32(30.0) * np.asarray(inputs["b2"], np.float32)).reshape(128, 1),
    }


def prep_core(pts, lay: Layout, Q):
    b = point_bins(pts)
    order = np.lexsort((b["zbin"], b["ybin"], b["x0"]))
    x0 = b["x0"][order]; y0 = b["y0"][order]; z0 = b["z0"][order]
    wx0 = b["wx0"][order]; wx1 = b["wx1"][order]
    wy0 = b["wy0"][order]; wy1 = b["wy1"][order]
    wz0 = b["wz0"][order]; wz1 = b["wz1"][order]
    ybin = b["ybin"][order]; zbin = b["zbin"][order]
    cell = b["cell"][order]

    gc = x0 * (NB * NB) + cell
    changes = np.empty(len(gc), bool); changes[0] = True
    changes[1:] = gc[1:] != gc[:-1]
    run_start = np.maximum.accumulate(np.where(changes, np.arange(len(gc)), 0))
    iic = np.arange(len(gc)) - run_start
    slot_local = lay.cell_off[x0, cell] + iic
    slots = lay.slot_off[x0] + slot_local

    M = np.zeros((128, lay.TOT), np.float16)
    # stationaries (same all cores)
    for g in range(NG):
        off = int(lay.seg_off[g])
        for w in range(NW):
            ws = lay.wslot[g, w]
            if ws >= 0:
                M[:, off + ws * 128: off + (ws + 1) * 128] = Q[g, w]

    seg0 = lay.seg_off[x0]

    def scatter_sample(r0, w0a, w1a, sbin, blk_off, rng_lo):
        """One sample (y or z).  r0 = row0 (y0 or z0), w*a = weights,
        sbin = bin array, blk_off[g, w], rng_lo[g, w] = slot range lo."""
        q, rm = np.divmod(r0, 64)
        interior = rm != 63
        # interior: 4 corners in window q
        for dr, wy in ((0, w0a), (1, w1a)):
            for col, wx in ((0, wx0), (1, wx1)):
                mask = interior
                rows = (rm + dr) * 2 + col
                cols = (seg0 + blk_off[x0, np.minimum(q, NW - 1)]
                        + slot_local - rng_lo[x0, np.minimum(q, NW - 1)])
                vals = (wy * wx).astype(np.float16)
                M[rows[mask], cols[mask]] = vals[mask]
        # straddle: row0 corner (row 63) in window q, row0+1 (row 0) in q+1
        st = ~interior
        for col, wx in ((0, wx0), (1, wx1)):
            rows = np.full_like(r0, 126 + col)      # 63*2+col
            cols = seg0 + blk_off[x0, np.minimum(q, NW - 1)] + slot_local \
                - rng_lo[x0, np.minimum(q, NW - 1)]
            vals = (w0a * wx).astype(np.float16)
            M[rows[st], cols[st]] = vals[st]
            q1 = np.minimum(q + 1, NW - 1)
            rows1 = np.full_like(r0, col)           # 0*2+col
            cols1 = seg0 + blk_off[x0, q1] + slot_local - rng_lo[x0, q1]
            vals1 = (w1a * wx).astype(np.float16)
            M[rows1[st], cols1[st]] = vals1[st]

    # y sample: block offset per (g, w); slot-range lo = ylo
    scatter_sample(y0, wy0, wy1, ybin, lay.yblk_off, lay.ylo)

    # z sample: block cols are per-run: local col = zblk_off + zrun_off[w, r]
    #           + (slot_local - cell_off[g, r*NB + blo])
    zq, zrm = np.divmod(z0, 64)
    z_interior = zrm != 63

    def zcols(wv):
        blo = np.maximum(2 * wv - 1, 0)
        base = lay.cell_off[x0, ybin * NB + blo]
        return (seg0 + lay.zblk_off[x0, wv] + lay.zrun_off[x0, wv, ybin]
                + slot_local - base)

    for dr, wz in ((0, wz0), (1, wz1)):
        for col, wx in ((0, wx0), (1, wx1)):
            mask = z_interior
            rows = (zrm + dr) * 2 + col
            cols = zcols(np.minimum(zq, NW - 1))
            vals = (wz * wx).astype(np.float16)
            M[rows[mask], cols[mask]] = vals[mask]
    st = ~z_interior
    for col, wx in ((0, wx0), (1, wx1)):
        rows = np.full_like(z0, 126 + col)
        cols = zcols(np.minimum(zq, NW - 1))
        vals = (wz0 * wx).astype(np.float16)
        M[rows[st], cols[st]] = vals[st]
        q1 = np.minimum(zq + 1, NW - 1)
        rows1 = np.full_like(z0, col)
        cols1 = zcols(q1)
        vals1 = (wz1 * wx).astype(np.float16)
        M[rows1[st], cols1[st]] = vals1[st]

    return {"m": M, "order": order, "slots": slots}


# ---------------------------------------------------------------- device

def build_nc(lay: Layout):
    nc = bacc.Bacc("TRN2", target_bir_lowering=False, debug=False,
                   enable_asserts=False, num_devices=N_CORES)

    m_d = nc.dram_tensor("m", [128, lay.TOT], F16, kind="ExternalInput")
    w1t_d = nc.dram_tensor("w1t", [128, 128], F16, kind="ExternalInput")
    w2t_d = nc.dram_tensor("w2t", [128, 128], F16, kind="ExternalInput")
    w3t_d = nc.dram_tensor("w3t", [128, 1], F16, kind="ExternalInput")
    b1s_d = nc.dram_tensor("b1s", [128, 1], F32, kind="ExternalInput")
    b2s_d = nc.dram_tensor("b2s", [128, 1], F32, kind="ExternalInput")
    out_d = nc.dram_tensor("out", [lay.out_len], F32, kind="ExternalOutput")

    Sin = mybir.ActivationFunctionType.Sin
    mult = mybir.AluOpType.mult

    with tile.TileContext(nc) as tc:
        with (
            tc.tile_pool(name="const", bufs=1) as cpool,
            tc.tile_pool(name="seg", bufs=6) as mpool,
            tc.tile_pool(name="ps_y", bufs=3, space="PSUM") as xypool,
            tc.tile_pool(name="ps_z", bufs=2, space="PSUM") as xzpool,
            tc.tile_pool(name="xz2", bufs=3) as xpool,
            tc.tile_pool(name="fw", bufs=3) as fpool,
            tc.tile_pool(name="ps_m", bufs=2, space="PSUM") as mlppool,
            tc.tile_pool(name="ps_3", bufs=1, space="PSUM") as p3pool,
            tc.tile_pool(name="hid", bufs=2) as hpool,
            tc.tile_pool(name="ob", bufs=2) as opool,
        ):
            def load_const(name, dram, shape, dtype):
                t = cpool.tile(shape, dtype, tag=name)
                nc.sync.dma_start(out=t[:], in_=dram.ap())
                return t

            w1t = load_const("w1t", w1t_d, [128, 128], F16)
            w2t = load_const("w2t", w2t_d, [128, 128], F16)
            w3t = load_const("w3t", w3t_d, [128, 1], F16)
            b1s = load_const("b1s", b1s_d, [128, 1], F32)
            b2s = load_const("b2s", b2s_d, [128, 1], F32)

            co = lay.cell_off
            # window-psum state: interp matmuls from all groups accumulate
            # directly into per-window psum tiles; sq/feat/MLP run once per
            # 512-slot window.
            wstate = {}

            def get_win(w):
                if w not in wstate:
                    xyw = xypool.tile([128, WIN], F32, tag="xy", name="xy")
                    xzw = xzpool.tile([128, WIN], F32, tag="xz", name="xz")
                    wstate[w] = {"xy": xyw, "xz": xzw,
                                 "armed_xy": False, "armed_xz": False}
                return wstate[w]

            def flush_part1(w, n):
                """sq + feat (ACT/DVE only — doesn't block the PE queue)."""
                ws = wstate.pop(w)
                xyw, xzw = ws["xy"], ws["xz"]
                xz2 = xpool.tile([128, WIN], F16, tag="xz2")
                nc.scalar.square(xz2[:, :n], xzw[:, :n])
                fw = fpool.tile([128, WIN], F16, tag="fw", name="fw")
                nc.vector.tensor_tensor(out=fw[:, :n], in0=xyw[:, :n],
                                        in1=xz2[:, :n], op=mult)
                return fw

            def flush_part2(w, n, fw):
                """MLP + output — deferred so PE always has interp work."""
                ps1 = mlppool.tile([128, WIN], F32, tag="ps")
                nc.tensor.matmul(ps1[:, :n], w1t[:], fw[:, :n], start=True, stop=True)
                h1 = hpool.tile([128, WIN], F16, tag="h1")
                nc.scalar.activation(h1[:, :n], ps1[:, :n], Sin, bias=b1s[:], scale=30.0)
                ps2 = mlppool.tile([128, WIN], F32, tag="ps")
                nc.tensor.matmul(ps2[:, :n], w2t[:], h1[:, :n], start=True, stop=True)
                h2 = hpool.tile([128, WIN], F16, tag="h2")
                nc.scalar.activation(h2[:, :n], ps2[:, :n], Sin, bias=b2s[:], scale=30.0)
                ps3 = p3pool.tile([1, WIN], F32, tag="p3")
                nc.tensor.matmul(ps3[:, :n], w3t[:], h2[:, :n], start=True, stop=True)
                ob = opool.tile([1, WIN], F32, tag="ob")
                nc.vector.tensor_scalar_add(out=ob[:, :n], in0=ps3[:, :n],
                                            scalar1=0.0)
                nc.sync.dma_start(out=out_d.ap()[w * WIN: w * WIN + n],
                                  in_=ob[:, :n])

            win_done = 0
            pending = []
            for gp in range(0, NG, 2):
                a = int(lay.seg_off[gp])
                b2 = int(lay.seg_off[min(gp + 2, NG)])
                if b2 == a:
                    continue
                st = mpool.tile([128, lay.seg_max_pair], F16, tag="seg",
                                name="seg")
                deng = (nc.sync, nc.gpsimd)[(gp // 2) % 2]
                deng.dma_start(out=st[:, :b2 - a], in_=m_d.ap()[:, a:b2])

                for g in (gp, gp + 1):
                    if g >= NG:
                        continue
                    S_g = int(lay.S_g[g])
                    if S_g == 0:
                        continue
                    rel = int(lay.seg_off[g] - a)
                    s0 = int(lay.slot_off[g])

                    def stat(w):
                        ws = int(lay.wslot[g, w])
                        return st[:, rel + ws * 128: rel + (ws + 1) * 128]

                    def mm(key, w, s_a, s_b, mcol):
                        # group-local slots [s_a, s_b) -> global, split per win
                        ga, gb = s0 + int(s_a), s0 + int(s_b)
                        mc = rel + int(mcol)
                        while ga < gb:
                            wi = ga // WIN
                            pe = min(gb, (wi + 1) * WIN)
                            wst = get_win(wi)
                            ps = wst[key]
                            la, lb = ga - wi * WIN, pe - wi * WIN
                            nc.tensor.matmul(
                                ps[:, la:lb], stat(w),
                                st[:, mc:mc + (pe - ga)],
                                start=not wst["armed_" + key], stop=True,
                                skip_group_check=True)
                            wst["armed_" + key] = True
                            mc += pe - ga
                            ga = pe

                    for w in range(NW):
                        if lay.ybw[g, w] > 0:
                            mm("xy", w, lay.ylo[g, w], lay.yhi[g, w],
                               lay.yblk_off[g, w])
                    for w in range(NW):
                        if lay.zbw[g, w] == 0:
                            continue
                        blo, bhi = win_bins(w)
                        runs = []
                        for r in range(NB):
                            s_a = int(co[g, r * NB + blo])
                            s_b = int(co[g, r * NB + bhi + 1])
                            if s_b <= s_a:
                                continue
                            bo = int(lay.zblk_off[g, w] + lay.zrun_off[g, w, r])
                            if runs and runs[-1][1] == s_a:
                                runs[-1][1] = s_b
                            else:
                                runs.append([s_a, s_b, bo])
                        for s_a, s_b, bo in runs:
                            mm("xz", w, s_a, s_b, bo)

                    gend = int(lay.slot_off[g + 1])
                    while (win_done + 1) * WIN <= gend:
                        fw = flush_part1(win_done, WIN)
                        flush_part2(win_done, WIN, fw)
                        win_done += 1
            if win_done * WIN < lay.S_total:
                n = lay.S_total - win_done * WIN
                fw = flush_part1(win_done, n)
                flush_part2(win_done, n, fw)

    nc.compile()
    return nc


_NC_CACHE = {}


def get_nc(lay: Layout):
    k = lay.key()
    if k not in _NC_CACHE:
        _NC_CACHE[k] = build_nc(lay)
    return _NC_CACHE[k]


LAST_RESULT = None


def kernel(_trace=False, **inputs):
    global LAST_RESULT
    from concourse.bass_utils import run_bass_kernel_spmd

    coords = np.asarray(inputs["coordinates"], np.float32).reshape(-1, 3)
    assert coords.shape[0] == N_TOTAL
    shared = prep_shared(inputs)
    Q = shared.pop("Q")
    b3 = np.float32(np.asarray(inputs["b3"], np.float32).reshape(-1)[0])

    # Global sort + round-robin deal: per-(group,cell) counts become
    # near-identical across cores, so the max-over-cores cap padding is
    # ~1.5% instead of ~11%.
    ball = point_bins(coords)
    order_all = np.lexsort((ball["zbin"], ball["ybin"], ball["x0"]))
    core_idx = [order_all[ci::N_CORES] for ci in range(N_CORES)]
    chunks = [coords[idx] for idx in core_idx]
    bins = [point_bins(p) for p in chunks]
    caps = np.stack([core_counts(b) for b in bins]).max(axis=0)
    lay = Layout(caps)
    nc = get_nc(lay)

    in_maps, metas = [], []
    for ci in range(N_CORES):
        m = prep_core(chunks[ci], lay, Q)
        metas.append(m)
        in_maps.append({**shared, "m": m["m"]})
    res = run_bass_kernel_spmd(nc, in_maps, core_ids=list(range(N_CORES)),
                               trace=_trace)
    LAST_RESULT = res
    out_full = np.empty(N_TOTAL, np.float32)
    for ci in range(N_CORES):
        r = np.asarray(res.results[ci]["out"], np.float32)
        m = metas[ci]
        chunk_out = np.empty(NPC, np.float32)
        chunk_out[m["order"]] = r[m["slots"]]
        out_full[core_idx[ci]] = chunk_out + b3
    return out_full.reshape(1, N_TOTAL, 1)
